# revision 18
# baseline (speedup 1.0000x reference)
"""CenterLoss kernel for Trainium2 (8 NeuronCores).

reference:
    gathered = centers[labels]            # [B, D] gather from [V, D]
    loss = sum((feat - gathered)**2) / B / 2

Sharding strategy (the hint's row-shard variant): samples are assigned to
cores BY LABEL RANGE — core k owns centers rows [k*12500, (k+1)*12500) and
receives exactly the (feat row, label) pairs whose label falls in its range
(plus padding). Local indices are then < 12500, which fits int16, unlocking
the DMAGatherAnt custom instruction (max 1024 indices per instruction,
HW-verified). Q7 descriptor generation runs at ~9ns/row either way, so the
gather is split into blocks with the SMALLEST block last (minimizes the
exposed transfer tail) and a tiny warm-up gather absorbs the ucode-library
IRAM cold-start while the input DMAs run.

Padding trick: pad slots use local label 0 and feat row := centers[k*12500]
so diff == 0 exactly and pads contribute nothing to the sum.

Layouts (HW-verified semantics from bass_interp/dma_gather ucode):
  - dma_gather dest: sample i -> partition i%128, column i//128.
  - dma_gather idxs: sample i -> partition i%16, column i//16, replicated
    across the 8 groups of 16 partitions.
Host pre-wraps feat and idx accordingly, so all device DMAs are contiguous.

Raw bacc engine blocks with manual semaphores (TileContext's drain + EVSEM
butterfly tail costs ~16us/launch). Compute instructions carry at most one
embedded semaphore wait (walrus codegen limit); standalone wait_ge is used.
"""

import math

import numpy as np

import concourse.bacc as bacc
from concourse import library_config, mybir

NUM_CLASSES = 100000
D = 256
B = 16384
N_CORES = 8
R = NUM_CLASSES // N_CORES  # 12500 centers rows per core
P = 128
MAX_GATHER_COLS = 8   # 1024 idx = dma_gather per-instruction limit
CHUNK_COLS = 4        # DVE/ACT chunk size (columns)

_CACHE = {}


def _col_blocks(J):
    """Split J columns into gather blocks of <=8 cols, smallest LAST."""
    blocks = []
    rem = J
    while rem > 0:
        take = min(MAX_GATHER_COLS, rem)
        blocks.append(take)
        rem -= take
    blocks.sort(reverse=True)
    return blocks


def _chunks_of(blocks):
    """Per gather block, the list of (block_idx, col_lo, col_hi) chunks."""
    chunks = []
    lo = 0
    for h, bc in enumerate(blocks):
        off = 0
        while off < bc:
            take = min(CHUNK_COLS, bc - off)
            chunks.append((h, lo + off, lo + off + take))
            off += take
        lo += bc
    return chunks


def build_nc(J):
    """J = gathered columns per partition (C = 128*J samples per core)."""
    C = P * J
    blocks = _col_blocks(J)
    chunks = _chunks_of(blocks)
    NCHUNKS = len(chunks)

    nc = bacc.Bacc(
        "TRN2",
        target_bir_lowering=False,
        enable_partition_id=False,
        monotonic_sem_count=0,
        dynamic_dma_scratch_size=65536,
    )
    feat = nc.declare_dram_parameter("feat", [P, J, D], mybir.dt.float32, isOutput=False)
    idx16 = nc.declare_dram_parameter("idx16", [P, C // 16], mybir.dt.int16, isOutput=False)
    slab = nc.declare_dram_parameter("slab", [R, D], mybir.dt.float32, isOutput=False)
    out = nc.declare_dram_parameter("out", [1, 1], mybir.dt.float32, isOutput=True)

    with (
        nc.semaphore("I") as I,   # idx dma done
        nc.semaphore("F") as F,   # feat dma done
        nc.semaphore("V") as V,   # DVE progress
        nc.semaphore("A") as A,   # ACT progress
        nc.semaphore("M") as M,   # PE matmul done
        nc.semaphore("O") as O,   # out dma done
        nc.sbuf_tensor("idx_sb", [P, C // 16], mybir.dt.int16) as idx_sb,
        nc.sbuf_tensor("warm_idx", [P, 1], mybir.dt.int16) as warm_idx,
        nc.sbuf_tensor("warm_dst", [P, 1, D], mybir.dt.float32) as warm_dst,
        nc.sbuf_tensor("feat_sb", [P, J, D], mybir.dt.float32) as feat_sb,
        nc.sbuf_tensor("gath_sb", [P, J, D], mybir.dt.float32) as gath_sb,
        nc.sbuf_tensor("diff_sb", [P, J, D], mybir.dt.float32) as diff_sb,
        nc.sbuf_tensor("acc_sb", [P, NCHUNKS], mybir.dt.float32) as acc_sb,
        nc.sbuf_tensor("red_sb", [P, 1], mybir.dt.float32) as red_sb,
        nc.sbuf_tensor("ones_sb", [P, 1], mybir.dt.float32) as ones_sb,
        nc.sbuf_tensor("res_sb", [1, 1], mybir.dt.float32) as res_sb,
        nc.psum_tensor("res_ps", [1, 1], mybir.dt.float32) as res_ps,
    ):
        G_sems = [nc.alloc_semaphore(f"G{h}") for h in range(len(blocks))]
        W = nc.alloc_semaphore("W")

        with nc.Block(no_gpsimd_drain=True) as block:

            @block.sync
            def _(sync):
                sync.dma_start(out=idx_sb[:, :], in_=idx16[:, :]).then_inc(I, 16)
                sync.dma_start(out=feat_sb[:, :, :], in_=feat[:, :, :]).then_inc(F, 16)
                sync.wait_ge(V, NCHUNKS + 3)
                sync.dma_start(out=out[:, :], in_=res_sb[:, :]).then_inc(O, 16)
                sync.wait_ge(O, 16)

            @block.gpsimd
            def _(gpsimd):
                gpsimd.load_library(library_config.mlp)
                # warm-up: pulls the gather ucode into IRAM during input DMAs
                gpsimd.memset(warm_idx[:, :], 0)
                gpsimd.dma_gather(
                    warm_dst[:, :, :], slab[:, :], warm_idx[:, :], 16, 16, D
                ).then_inc(W, 16)
                gpsimd.wait_ge(I, 16)
                lo = 0
                for h, bc in enumerate(blocks):
                    bs = bc * P
                    gpsimd.dma_gather(
                        gath_sb[:, lo : lo + bc, :],
                        slab[:, :],
                        idx_sb[:, lo * (P // 16) : (lo + bc) * (P // 16)],
                        bs,
                        bs,
                        D,
                    ).then_inc(G_sems[h], 16)
                    lo += bc

            @block.vector
            def _(vector):
                vector.memset(ones_sb[:, :], 1.0).then_inc(V, 1)  # V=1
                vector.wait_ge(F, 16)
                for t, (h, lo, hi) in enumerate(chunks):
                    vector.wait_ge(G_sems[h], 16)
                    vector.tensor_sub(
                        out=diff_sb[:, lo:hi, :],
                        in0=feat_sb[:, lo:hi, :],
                        in1=gath_sb[:, lo:hi, :],
                    ).then_inc(V, 1)  # V = 2+t
                vector.wait_ge(A, NCHUNKS)
                vector.tensor_reduce(
                    out=red_sb[:, :],
                    in_=acc_sb[:, :],
                    axis=mybir.AxisListType.X,
                    op=mybir.AluOpType.add,
                ).then_inc(V, 1)  # V = NCHUNKS+2
                vector.wait_ge(M, 1)
                vector.tensor_copy(out=res_sb[:, :], in_=res_ps[:, :]).then_inc(
                    V, 1
                )  # V = NCHUNKS+3

            @block.scalar
            def _(scalar):
                for t, (h, lo, hi) in enumerate(chunks):
                    scalar.wait_ge(V, 2 + t)
                    scalar.activation(
                        diff_sb[:, lo:hi, :],
                        diff_sb[:, lo:hi, :],
                        mybir.ActivationFunctionType.Square,
                        accum_out=acc_sb[:, t : t + 1],
                    ).then_inc(A, 1)

            @block.tensor
            def _(tensor):
                tensor.wait_ge(V, NCHUNKS + 2)
                tensor.matmul(
                    out=res_ps[:, :],
                    lhsT=ones_sb[:, :],
                    rhs=red_sb[:, :],
                    start=True,
                    stop=True,
                ).then_inc(M, 1)

    nc.compile()
    return nc


def _get_nc(J):
    key = ("nc", J)
    if key not in _CACHE:
        _CACHE[key] = build_nc(J)
    return _CACHE[key]


def _wrap_feat(feat_pad, J):
    # sample i -> partition i%128, column i//128, per gather block
    blocks = _col_blocks(J)
    out = []
    lo = 0
    for bc in blocks:
        bs = bc * P
        out.append(
            feat_pad[lo : lo + bs].reshape(bc, P, D).transpose(1, 0, 2)
        )
        lo += bs
    return np.ascontiguousarray(np.concatenate(out, axis=1))


def _wrap_idx(ll_pad, J):
    # sample i -> partition i%16, column i//16, replicated to 8 groups
    blocks = _col_blocks(J)
    out = []
    lo = 0
    for bc in blocks:
        bs = bc * P
        out.append(
            np.tile(ll_pad[lo : lo + bs].reshape(bs // 16, 16).T, (8, 1))
        )
        lo += bs
    return np.ascontiguousarray(np.concatenate(out, axis=1))


def make_in_maps(feat, labels, centers):
    feat = np.ascontiguousarray(np.asarray(feat, dtype=np.float32))
    centers = np.ascontiguousarray(np.asarray(centers, dtype=np.float32))
    labels_i64 = np.asarray(labels).astype(np.int64)
    assert feat.shape == (B, D) and labels_i64.shape == (B,)
    assert centers.shape == (NUM_CLASSES, D)

    core_of = labels_i64 // R
    order = np.argsort(core_of, kind="stable")
    counts = np.bincount(core_of, minlength=N_CORES)
    J = max(2, math.ceil(counts.max() / P))
    C = P * J

    in_maps = []
    start = 0
    for k in range(N_CORES):
        n_k = int(counts[k])
        idxs = order[start : start + n_k]
        start += n_k
        ll_pad = np.zeros(C, dtype=np.int16)
        ll_pad[:n_k] = (labels_i64[idxs] - k * R).astype(np.int16)
        feat_pad = np.empty((C, D), dtype=np.float32)
        feat_pad[:n_k] = feat[idxs]
        feat_pad[n_k:] = centers[k * R]  # pads: diff == 0 exactly
        in_maps.append(
            {
                "feat": _wrap_feat(feat_pad, J),
                "idx16": _wrap_idx(ll_pad, J),
                "slab": centers[k * R : (k + 1) * R],
            }
        )
    return J, in_maps


def kernel(feat, labels, centers):
    from concourse.bass_utils import run_bass_kernel_spmd

    J, in_maps = make_in_maps(feat, labels, centers)
    nc = _get_nc(J)
    res = run_bass_kernel_spmd(nc, in_maps, list(range(N_CORES)))
    total = float(sum(float(r["out"][0, 0]) for r in res.results))
    return np.float32(total / B / 2.0)


# revision 19
# speedup vs baseline: 1.2166x; 1.2166x over previous
"""CenterLoss kernel for Trainium2 (8 NeuronCores, data-parallel over batch).

reference:
    gathered = centers[labels]            # [B, D] gather from [V, D]
    loss = sum((feat - gathered)**2) / B / 2

Sharding: feat/labels split along batch across 8 cores; centers replicated
(each core only reads the rows its labels hit). Each core computes a scalar
partial sum on-device; the host adds the 8 partials and normalizes.

Per-core dataflow (HW-verified design notes):
  - The gather is 16 x indirect_dma_start with [P, 1] offset APs (one row
    per partition). Multi-column/flat offset APs scramble or hang on HW.
    Q7 descriptor generation costs ~1.1us per 128-row gather, which is the
    kernel's critical path (~22us); the DMAGatherAnt alternative has the
    same per-row descriptor rate but adds a ~13.5us ucode-library IRAM load,
    so it loses on a cold execution.
  - Raw bacc engine blocks with manual semaphores: TileContext's kernel-tail
    drain + EVSEM butterfly adds ~16us/launch.
  - Gathers that feed one compute chunk share that chunk's semaphore (DMA
    completions are unordered; per-chunk sems avoid threshold races).
  - Chunk sizes shrink (6,6,3,1 tiles) so the work remaining after the last
    gather lands is tiny.
  - Output is reduced on-device to [1,1] (PE matmul with ones): a [128,1]
    column DMA costs ~7us in tiny-descriptor completion latency.
  - Compute instructions carry at most ONE embedded semaphore wait (walrus
    codegen limit); standalone wait_ge instructions are used instead.
"""

import numpy as np

import concourse.bacc as bacc
import concourse.bass as bass
from concourse import mybir

NUM_CLASSES = 100000
D = 256
B = 16384
N_CORES = 8
B_SHARD = B // N_CORES  # 2048
P = 128
T = B_SHARD // P  # 16 gathered tiles (columns) per core
CHUNK_TILES = [6, 6, 3, 1]  # decreasing -> minimal post-gather tail
NCHUNKS = len(CHUNK_TILES)

_CACHE = {}


def build_nc():
    assert sum(CHUNK_TILES) == T
    chunk_of_tile = []
    for c, n in enumerate(CHUNK_TILES):
        chunk_of_tile += [c] * n
    bounds = np.cumsum([0] + CHUNK_TILES)

    nc = bacc.Bacc(
        "TRN2",
        target_bir_lowering=False,
        enable_partition_id=False,
        monotonic_sem_count=0,
        dynamic_dma_scratch_size=65536,
    )
    feat = nc.declare_dram_parameter("feat", [P, T, D], mybir.dt.float32, isOutput=False)
    labels = nc.declare_dram_parameter("labels", [P, T], mybir.dt.int32, isOutput=False)
    centers = nc.declare_dram_parameter(
        "centers", [NUM_CLASSES, D], mybir.dt.float32, isOutput=False
    )
    out = nc.declare_dram_parameter("out", [1, 1], mybir.dt.float32, isOutput=True)

    with (
        nc.semaphore("L") as L,   # labels dma done
        nc.semaphore("F") as F,   # feat dma done
        nc.semaphore("V") as V,   # DVE progress
        nc.semaphore("A") as A,   # ACT progress
        nc.semaphore("M") as M,   # PE matmul done
        nc.semaphore("O") as O,   # out dma done
        nc.sbuf_tensor("labels_sb", [P, T], mybir.dt.int32) as labels_sb,
        nc.sbuf_tensor("feat_sb", [P, T, D], mybir.dt.float32) as feat_sb,
        nc.sbuf_tensor("gath_sb", [P, T, D], mybir.dt.float32) as gath_sb,
        nc.sbuf_tensor("diff_sb", [P, T, D], mybir.dt.float32) as diff_sb,
        nc.sbuf_tensor("acc_sb", [P, NCHUNKS], mybir.dt.float32) as acc_sb,
        nc.sbuf_tensor("red_sb", [P, 1], mybir.dt.float32) as red_sb,
        nc.sbuf_tensor("ones_sb", [P, 1], mybir.dt.float32) as ones_sb,
        nc.sbuf_tensor("res_sb", [1, 1], mybir.dt.float32) as res_sb,
        nc.psum_tensor("res_ps", [1, 1], mybir.dt.float32) as res_ps,
    ):
        G_sems = [nc.alloc_semaphore(f"G{c}") for c in range(NCHUNKS)]

        with nc.Block(no_gpsimd_drain=True) as block:

            @block.sync
            def _(sync):
                sync.dma_start(out=feat_sb[:, :, :], in_=feat[:, :, :]).then_inc(F, 16)
                sync.wait_ge(V, NCHUNKS + 3)
                sync.dma_start(out=out[:, :], in_=res_sb[:, :]).then_inc(O, 16)
                sync.wait_ge(O, 16)

            @block.gpsimd
            def _(gpsimd):
                # gpsimd loads its own offsets: gathers start without waiting
                # on the sync engine's (bigger) feat DMA.
                gpsimd.dma_start(out=labels_sb[:, :], in_=labels[:, :]).then_inc(L, 16)
                gpsimd.wait_ge(L, 16)
                for t in range(T):
                    gpsimd.indirect_dma_start(
                        out=gath_sb[:, t, :],
                        out_offset=None,
                        in_=centers[:],
                        in_offset=bass.IndirectOffsetOnAxis(
                            ap=labels_sb[:, t : t + 1], axis=0
                        ),
                    ).then_inc(G_sems[chunk_of_tile[t]], 16)

            @block.vector
            def _(vector):
                vector.memset(ones_sb[:, :], 1.0).then_inc(V, 1)  # V=1
                vector.wait_ge(F, 16)
                for c in range(NCHUNKS):
                    lo, hi = int(bounds[c]), int(bounds[c + 1])
                    vector.wait_ge(G_sems[c], 16 * CHUNK_TILES[c])
                    vector.tensor_sub(
                        out=diff_sb[:, lo:hi, :],
                        in0=feat_sb[:, lo:hi, :],
                        in1=gath_sb[:, lo:hi, :],
                    ).then_inc(V, 1)  # V = 2+c
                vector.wait_ge(A, NCHUNKS)
                vector.tensor_reduce(
                    out=red_sb[:, :],
                    in_=acc_sb[:, :],
                    axis=mybir.AxisListType.X,
                    op=mybir.AluOpType.add,
                ).then_inc(V, 1)  # V = NCHUNKS+2
                vector.wait_ge(M, 1)
                vector.tensor_copy(out=res_sb[:, :], in_=res_ps[:, :]).then_inc(
                    V, 1
                )  # V = NCHUNKS+3

            @block.scalar
            def _(scalar):
                for c in range(NCHUNKS):
                    lo, hi = int(bounds[c]), int(bounds[c + 1])
                    scalar.wait_ge(V, 2 + c)
                    scalar.activation(
                        diff_sb[:, lo:hi, :],
                        diff_sb[:, lo:hi, :],
                        mybir.ActivationFunctionType.Square,
                        accum_out=acc_sb[:, c : c + 1],
                    ).then_inc(A, 1)

            @block.tensor
            def _(tensor):
                tensor.wait_ge(V, NCHUNKS + 2)
                tensor.matmul(
                    out=res_ps[:, :],
                    lhsT=ones_sb[:, :],
                    rhs=red_sb[:, :],
                    start=True,
                    stop=True,
                ).then_inc(M, 1)

    nc.compile()
    return nc


def _get_nc():
    if "nc" not in _CACHE:
        _CACHE["nc"] = build_nc()
    return _CACHE["nc"]


def make_in_maps(feat, labels, centers):
    feat = np.ascontiguousarray(np.asarray(feat, dtype=np.float32))
    centers = np.ascontiguousarray(np.asarray(centers, dtype=np.float32))
    labels_i32 = np.asarray(labels).astype(np.int32)
    assert feat.shape == (B, D) and labels_i32.shape == (B,)
    assert centers.shape == (NUM_CLASSES, D)
    in_maps = []
    for c in range(N_CORES):
        lo, hi = c * B_SHARD, (c + 1) * B_SHARD
        in_maps.append(
            {
                "feat": feat[lo:hi].reshape(P, T, D),
                "labels": labels_i32[lo:hi].reshape(P, T),
                "centers": centers,
            }
        )
    return in_maps


def kernel(feat, labels, centers):
    from concourse.bass_utils import run_bass_kernel_spmd

    nc = _get_nc()
    in_maps = make_in_maps(feat, labels, centers)
    res = run_bass_kernel_spmd(nc, in_maps, list(range(N_CORES)))
    total = float(sum(float(r["out"][0, 0]) for r in res.results))
    return np.float32(total / B / 2.0)


# revision 20
# speedup vs baseline: 1.3048x; 1.0724x over previous
"""CenterLoss kernel for Trainium2 (8 NeuronCores, data-parallel over batch).

reference:
    gathered = centers[labels]            # [B, D] gather from [V, D]
    loss = sum((feat - gathered)**2) / B / 2

Sharding: feat/labels split along batch across 8 cores; centers replicated
(each core only reads the rows its labels hit). Each core computes a scalar
partial sum on-device; the host adds the 8 partials and normalizes.

Per-core dataflow (HW-verified design notes):
  - The gather is 16 x indirect_dma_start with [P, 1] offset APs (one row
    per partition). Multi-column/flat offset APs scramble or hang on HW.
    Q7 descriptor generation costs ~1.1us per 128-row gather, which is the
    kernel's critical path (~22us); the DMAGatherAnt alternative has the
    same per-row descriptor rate but adds a ~13.5us ucode-library IRAM load,
    so it loses on a cold execution.
  - Raw bacc engine blocks with manual semaphores: TileContext's kernel-tail
    drain + EVSEM butterfly adds ~16us/launch.
  - Gathers that feed one compute chunk share that chunk's semaphore (DMA
    completions are unordered; per-chunk sems avoid threshold races).
  - Chunk sizes shrink (6,6,3,1 tiles) so the work remaining after the last
    gather lands is tiny.
  - Output is reduced on-device to [1,1] (PE matmul with ones): a [128,1]
    column DMA costs ~7us in tiny-descriptor completion latency.
  - Compute instructions carry at most ONE embedded semaphore wait (walrus
    codegen limit); standalone wait_ge instructions are used instead.
"""

import numpy as np

import concourse.bacc as bacc
import concourse.bass as bass
from concourse import mybir

NUM_CLASSES = 100000
D = 256
B = 16384
N_CORES = 8
B_SHARD = B // N_CORES  # 2048
P = 128
T = B_SHARD // P  # 16 gathered tiles (columns) per core
CHUNK_TILES = [6, 6, 3, 1]  # decreasing -> minimal post-gather tail
NCHUNKS = len(CHUNK_TILES)

_CACHE = {}


def build_nc():
    assert sum(CHUNK_TILES) == T
    chunk_of_tile = []
    for c, n in enumerate(CHUNK_TILES):
        chunk_of_tile += [c] * n
    bounds = np.cumsum([0] + CHUNK_TILES)

    nc = bacc.Bacc(
        "TRN2",
        target_bir_lowering=False,
        enable_partition_id=False,
        monotonic_sem_count=0,
        dynamic_dma_scratch_size=65536,
    )
    feat = nc.declare_dram_parameter("feat", [P, T, D], mybir.dt.float32, isOutput=False)
    labels = nc.declare_dram_parameter("labels", [P, T], mybir.dt.int32, isOutput=False)
    centers = nc.declare_dram_parameter(
        "centers", [NUM_CLASSES, D], mybir.dt.float32, isOutput=False
    )
    out = nc.declare_dram_parameter("out", [1, 1], mybir.dt.float32, isOutput=True)

    with (
        nc.semaphore("L") as L,   # labels dma done
        nc.semaphore("F") as F,   # feat dma done
        nc.semaphore("V") as V,   # DVE progress
        nc.semaphore("A") as A,   # ACT progress
        nc.semaphore("M") as M,   # PE matmul done
        nc.semaphore("O") as O,   # out dma done
        nc.sbuf_tensor("labels_sb", [P, T], mybir.dt.int32) as labels_sb,
        nc.sbuf_tensor("feat_sb", [P, T, D], mybir.dt.float32) as feat_sb,
        nc.sbuf_tensor("gath_sb", [P, T, D], mybir.dt.float32) as gath_sb,
        nc.sbuf_tensor("diff_sb", [P, T, D], mybir.dt.float32) as diff_sb,
        nc.sbuf_tensor("acc_sb", [P, NCHUNKS], mybir.dt.float32) as acc_sb,
        nc.sbuf_tensor("red_sb", [P, 1], mybir.dt.float32) as red_sb,
        nc.sbuf_tensor("ones_sb", [P, 1], mybir.dt.float32) as ones_sb,
        nc.sbuf_tensor("res_sb", [1, 1], mybir.dt.float32) as res_sb,
        nc.psum_tensor("res_ps", [1, 1], mybir.dt.float32) as res_ps,
    ):
        G_sems = [nc.alloc_semaphore(f"G{c}") for c in range(NCHUNKS)]

        with nc.Block(no_gpsimd_drain=True) as block:

            @block.sync
            def _(sync):
                # labels first: tiny HWDGE transfer whose completion gates all
                # gathers (SWDGE-issued tiny descriptors take ~7us to signal).
                sync.dma_start(out=labels_sb[:, :], in_=labels[:, :]).then_inc(L, 16)
                sync.dma_start(out=feat_sb[:, :, :], in_=feat[:, :, :]).then_inc(F, 16)
                sync.wait_ge(V, NCHUNKS + 3)
                sync.dma_start(out=out[:, :], in_=res_sb[:, :]).then_inc(O, 16)
                sync.wait_ge(O, 16)

            @block.gpsimd
            def _(gpsimd):
                gpsimd.wait_ge(L, 16)
                for t in range(T):
                    gpsimd.indirect_dma_start(
                        out=gath_sb[:, t, :],
                        out_offset=None,
                        in_=centers[:],
                        in_offset=bass.IndirectOffsetOnAxis(
                            ap=labels_sb[:, t : t + 1], axis=0
                        ),
                    ).then_inc(G_sems[chunk_of_tile[t]], 16)

            @block.vector
            def _(vector):
                vector.memset(ones_sb[:, :], 1.0).then_inc(V, 1)  # V=1
                vector.wait_ge(F, 16)
                for c in range(NCHUNKS):
                    lo, hi = int(bounds[c]), int(bounds[c + 1])
                    vector.wait_ge(G_sems[c], 16 * CHUNK_TILES[c])
                    vector.tensor_sub(
                        out=diff_sb[:, lo:hi, :],
                        in0=feat_sb[:, lo:hi, :],
                        in1=gath_sb[:, lo:hi, :],
                    ).then_inc(V, 1)  # V = 2+c
                vector.wait_ge(A, NCHUNKS)
                vector.tensor_reduce(
                    out=red_sb[:, :],
                    in_=acc_sb[:, :],
                    axis=mybir.AxisListType.X,
                    op=mybir.AluOpType.add,
                ).then_inc(V, 1)  # V = NCHUNKS+2
                vector.wait_ge(M, 1)
                vector.tensor_copy(out=res_sb[:, :], in_=res_ps[:, :]).then_inc(
                    V, 1
                )  # V = NCHUNKS+3

            @block.scalar
            def _(scalar):
                for c in range(NCHUNKS):
                    lo, hi = int(bounds[c]), int(bounds[c + 1])
                    scalar.wait_ge(V, 2 + c)
                    scalar.activation(
                        diff_sb[:, lo:hi, :],
                        diff_sb[:, lo:hi, :],
                        mybir.ActivationFunctionType.Square,
                        accum_out=acc_sb[:, c : c + 1],
                    ).then_inc(A, 1)

            @block.tensor
            def _(tensor):
                tensor.wait_ge(V, NCHUNKS + 2)
                tensor.matmul(
                    out=res_ps[:, :],
                    lhsT=ones_sb[:, :],
                    rhs=red_sb[:, :],
                    start=True,
                    stop=True,
                ).then_inc(M, 1)

    nc.compile()
    return nc


def _get_nc():
    if "nc" not in _CACHE:
        _CACHE["nc"] = build_nc()
    return _CACHE["nc"]


def make_in_maps(feat, labels, centers):
    feat = np.ascontiguousarray(np.asarray(feat, dtype=np.float32))
    centers = np.ascontiguousarray(np.asarray(centers, dtype=np.float32))
    labels_i32 = np.asarray(labels).astype(np.int32)
    assert feat.shape == (B, D) and labels_i32.shape == (B,)
    assert centers.shape == (NUM_CLASSES, D)
    in_maps = []
    for c in range(N_CORES):
        lo, hi = c * B_SHARD, (c + 1) * B_SHARD
        in_maps.append(
            {
                "feat": feat[lo:hi].reshape(P, T, D),
                "labels": labels_i32[lo:hi].reshape(P, T),
                "centers": centers,
            }
        )
    return in_maps


def kernel(feat, labels, centers):
    from concourse.bass_utils import run_bass_kernel_spmd

    nc = _get_nc()
    in_maps = make_in_maps(feat, labels, centers)
    res = run_bass_kernel_spmd(nc, in_maps, list(range(N_CORES)))
    total = float(sum(float(r["out"][0, 0]) for r in res.results))
    return np.float32(total / B / 2.0)


# revision 21
# speedup vs baseline: 1.3882x; 1.0640x over previous
"""CenterLoss kernel for Trainium2 (8 NeuronCores, data-parallel over batch).

reference:
    gathered = centers[labels]            # [B, D] gather from [V, D]
    loss = sum((feat - gathered)**2) / B / 2

Sharding: feat/labels split along batch across 8 cores; centers replicated
(each core only reads the rows its labels hit). Each core computes a scalar
partial sum on-device; the host adds the 8 partials and normalizes.

Per-core dataflow (HW-verified design notes):
  - The gather is 16 x indirect_dma_start with [P, 1] offset APs (one row
    per partition). Multi-column/flat offset APs scramble or hang on HW.
    Q7 descriptor generation costs ~1.1us per 128-row gather, which is the
    kernel's critical path (~22us); the DMAGatherAnt alternative has the
    same per-row descriptor rate but adds a ~13.5us ucode-library IRAM load,
    so it loses on a cold execution.
  - Raw bacc engine blocks with manual semaphores: TileContext's kernel-tail
    drain + EVSEM butterfly adds ~16us/launch.
  - Gathers that feed one compute chunk share that chunk's semaphore (DMA
    completions are unordered; per-chunk sems avoid threshold races).
  - Chunk sizes shrink (6,6,3,1 tiles) so the work remaining after the last
    gather lands is tiny.
  - Output is reduced on-device to [1,1] (PE matmul with ones): a [128,1]
    column DMA costs ~7us in tiny-descriptor completion latency.
  - Compute instructions carry at most ONE embedded semaphore wait (walrus
    codegen limit); standalone wait_ge instructions are used instead.
"""

import numpy as np

import concourse.bacc as bacc
import concourse.bass as bass
from concourse import mybir

NUM_CLASSES = 100000
D = 256
B = 16384
N_CORES = 8
B_SHARD = B // N_CORES  # 2048
P = 128
T = B_SHARD // P  # 16 gathered tiles (columns) per core
CHUNK_TILES = [6, 6, 3, 1]  # decreasing -> minimal post-gather tail
NCHUNKS = len(CHUNK_TILES)

_CACHE = {}


def build_nc():
    assert sum(CHUNK_TILES) == T
    chunk_of_tile = []
    for c, n in enumerate(CHUNK_TILES):
        chunk_of_tile += [c] * n
    bounds = np.cumsum([0] + CHUNK_TILES)

    nc = bacc.Bacc(
        "TRN2",
        target_bir_lowering=False,
        enable_partition_id=False,
        monotonic_sem_count=0,
        dynamic_dma_scratch_size=65536,
    )
    feat = nc.declare_dram_parameter("feat", [P, T, D], mybir.dt.float32, isOutput=False)
    labels = nc.declare_dram_parameter("labels", [P, T], mybir.dt.int32, isOutput=False)
    centers = nc.declare_dram_parameter(
        "centers", [NUM_CLASSES, D], mybir.dt.float32, isOutput=False
    )
    out = nc.declare_dram_parameter("out", [1, 1], mybir.dt.float32, isOutput=True)

    with (
        nc.semaphore("L") as L,   # labels dma done
        nc.semaphore("F") as F,   # feat dma done
        nc.semaphore("V") as V,   # DVE progress
        nc.semaphore("A") as A,   # ACT progress
        nc.semaphore("M") as M,   # PE matmul done
        nc.semaphore("O") as O,   # out dma done
        nc.sbuf_tensor("labels_sb", [P, T], mybir.dt.int32) as labels_sb,
        nc.sbuf_tensor("feat_sb", [P, T, D], mybir.dt.float32) as feat_sb,
        nc.sbuf_tensor("gath_sb", [P, T, D], mybir.dt.float32) as gath_sb,
        nc.sbuf_tensor("diff_sb", [P, T, D], mybir.dt.float32) as diff_sb,
        nc.sbuf_tensor("acc_sb", [P, NCHUNKS], mybir.dt.float32) as acc_sb,
        nc.sbuf_tensor("red_sb", [P, 1], mybir.dt.float32) as red_sb,
        nc.sbuf_tensor("ones_sb", [P, 1], mybir.dt.float32) as ones_sb,
        nc.sbuf_tensor("res_sb", [1, 1], mybir.dt.float32) as res_sb,
        nc.psum_tensor("res_ps", [1, 1], mybir.dt.float32) as res_ps,
    ):
        G_sems = [nc.alloc_semaphore(f"G{c}") for c in range(NCHUNKS)]

        with nc.Block(no_gpsimd_drain=True) as block:

            @block.sync
            def _(sync):
                # labels first: tiny HWDGE transfer whose completion gates all
                # gathers (SWDGE-issued tiny descriptors take ~7us to signal).
                sync.dma_start(out=labels_sb[:, :], in_=labels[:, :]).then_inc(L, 16)
                sync.dma_start(out=feat_sb[:, :, :], in_=feat[:, :, :]).then_inc(F, 16)
                sync.wait_ge(V, NCHUNKS + 3)
                # no wait on O: the Block-exit DRAIN on sync already covers
                # the in-flight 4B output DMA, overlapped with the epilogue.
                sync.dma_start(out=out[:, :], in_=res_sb[:, :]).then_inc(O, 16)

            @block.gpsimd
            def _(gpsimd):
                gpsimd.wait_ge(L, 16)
                for t in range(T):
                    gpsimd.indirect_dma_start(
                        out=gath_sb[:, t, :],
                        out_offset=None,
                        in_=centers[:],
                        in_offset=bass.IndirectOffsetOnAxis(
                            ap=labels_sb[:, t : t + 1], axis=0
                        ),
                    ).then_inc(G_sems[chunk_of_tile[t]], 16)

            @block.vector
            def _(vector):
                vector.memset(ones_sb[:, :], 1.0).then_inc(V, 1)  # V=1
                vector.wait_ge(F, 16)
                for c in range(NCHUNKS):
                    lo, hi = int(bounds[c]), int(bounds[c + 1])
                    vector.wait_ge(G_sems[c], 16 * CHUNK_TILES[c])
                    vector.tensor_sub(
                        out=diff_sb[:, lo:hi, :],
                        in0=feat_sb[:, lo:hi, :],
                        in1=gath_sb[:, lo:hi, :],
                    ).then_inc(V, 1)  # V = 2+c
                vector.wait_ge(A, NCHUNKS)
                vector.tensor_reduce(
                    out=red_sb[:, :],
                    in_=acc_sb[:, :],
                    axis=mybir.AxisListType.X,
                    op=mybir.AluOpType.add,
                ).then_inc(V, 1)  # V = NCHUNKS+2
                vector.wait_ge(M, 1)
                vector.tensor_copy(out=res_sb[:, :], in_=res_ps[:, :]).then_inc(
                    V, 1
                )  # V = NCHUNKS+3

            @block.scalar
            def _(scalar):
                for c in range(NCHUNKS):
                    lo, hi = int(bounds[c]), int(bounds[c + 1])
                    scalar.wait_ge(V, 2 + c)
                    scalar.activation(
                        diff_sb[:, lo:hi, :],
                        diff_sb[:, lo:hi, :],
                        mybir.ActivationFunctionType.Square,
                        accum_out=acc_sb[:, c : c + 1],
                    ).then_inc(A, 1)

            @block.tensor
            def _(tensor):
                tensor.wait_ge(V, NCHUNKS + 2)
                tensor.matmul(
                    out=res_ps[:, :],
                    lhsT=ones_sb[:, :],
                    rhs=red_sb[:, :],
                    start=True,
                    stop=True,
                ).then_inc(M, 1)

    nc.compile()
    return nc


def _get_nc():
    if "nc" not in _CACHE:
        _CACHE["nc"] = build_nc()
    return _CACHE["nc"]


def make_in_maps(feat, labels, centers):
    feat = np.ascontiguousarray(np.asarray(feat, dtype=np.float32))
    centers = np.ascontiguousarray(np.asarray(centers, dtype=np.float32))
    labels_i32 = np.asarray(labels).astype(np.int32)
    assert feat.shape == (B, D) and labels_i32.shape == (B,)
    assert centers.shape == (NUM_CLASSES, D)
    in_maps = []
    for c in range(N_CORES):
        lo, hi = c * B_SHARD, (c + 1) * B_SHARD
        in_maps.append(
            {
                "feat": feat[lo:hi].reshape(P, T, D),
                "labels": labels_i32[lo:hi].reshape(P, T),
                "centers": centers,
            }
        )
    return in_maps


def kernel(feat, labels, centers):
    from concourse.bass_utils import run_bass_kernel_spmd

    nc = _get_nc()
    in_maps = make_in_maps(feat, labels, centers)
    res = run_bass_kernel_spmd(nc, in_maps, list(range(N_CORES)))
    total = float(sum(float(r["out"][0, 0]) for r in res.results))
    return np.float32(total / B / 2.0)
